# revision 5
# baseline (speedup 1.0000x reference)
"""ArcFace logits on 8 Trainium2 NeuronCores (Bass, raw engine streams).

out[n, c] = S * cos(theta_nc + M * [c == labels[n]]),  cos from L2-normalized
embeddings [1024, 512] x weight [100000, 512].

Strategy: model-parallel over the class dim (partial-FC).  Classes are
padded/permuted on the host so that every core gets 12800 columns and its
128 label hits land on the diagonal of the first 128x128 output block.
That makes the compiled graph identical on all 8 cores and fully
label-independent: the margin fix is a cheap diagonal extract/rewrite with
an identity mask.  The host only moves data (transpose / permute / gather),
all FLOPs (normalization, matmul, margin trig) run on device.

Matmuls run in float32r (full-rate fp32, ~1.5e-4 rel err).  1/sqrt uses the
Ln/Exp activation tables (one table set, no reloads).
"""

import math

import numpy as np

import concourse.bass as bass
import concourse.mybir as mybir
from concourse.bass_utils import run_bass_kernel_spmd

AF = mybir.ActivationFunctionType
OP = mybir.AluOpType
F32 = mybir.dt.float32
F32R = mybir.dt.float32r

S = 30.0
MARGIN = 0.5
N, D, C = 1024, 512, 100000

NCORES = 8
CS = 12800            # classes per core (padded: 8 * 12800 = 102400)
CPAD = NCORES * CS
F = 512               # matmul free dim / class chunk width
NCHUNK = CS // F      # 25
KD = D // 128         # 4 contraction sub-tiles
NB = N // 128         # 8 row blocks
NTILES = NCHUNK * NB  # 200 output tiles per core
NPS = 6               # main PSUM bank rotation
NWT = 4               # wT chunk buffers
NOUT = 24             # out_sb rotation (3 chunks)

COSM = float(math.cos(MARGIN))
SINM = float(math.sin(MARGIN))


def build_graph():
    nc = bass.Bass(target_bir_lowering=False)

    eT_ext = nc.declare_dram_parameter("eT", [D, N], F32, isOutput=False)
    wT_ext = nc.declare_dram_parameter("wT", [D, CS], F32, isOutput=False)
    ones_ext = nc.declare_dram_parameter("ones", [128], F32, isOutput=False)
    ident_ext = nc.declare_dram_parameter("ident", [128, 128], F32, isOutput=False)
    out_ext = nc.declare_dram_parameter("out", [N, CS], F32, isOutput=True)

    import contextlib

    ctx = contextlib.ExitStack()
    sb = lambda name, shape, dt=F32: ctx.enter_context(nc.sbuf_tensor(name, shape, dt))
    ps = lambda name: ctx.enter_context(nc.psum_tensor(name, [128, F], F32))
    sem = lambda name: ctx.enter_context(nc.semaphore(name))

    with ctx:
        # --- SBUF ---
        eT_sb = sb("eT_sb", [128, KD, N])            # raw e^T
        esq = sb("esq", [128, KD, N], F32R)          # e^2 (rounded)
        eTn = sb("eTn", [128, KD, N], F32R)          # S * e / ||e||
        rsqe_row = sb("rsqe_row", [1, N], F32R)
        rsqe_bc = sb("rsqe_bc", [128, N])
        wt = [sb(f"wt{b}", [128, KD, F], F32R) for b in range(NWT)]
        wsq = [sb(f"wsq{b}", [128, KD, F], F32R) for b in range(2)]
        lnw_tmp = sb("lnw_tmp", [1, F])
        rsqw_row = [sb(f"rsqw_row{b}", [1, F], F32R) for b in range(2)]
        rsqw_bc = [sb(f"rsqw_bc{b}", [128, F]) for b in range(2)]
        out_sb = [sb(f"out_sb{b}", [128, F]) for b in range(NOUT)]
        ones128 = sb("ones128", [128, 1], F32R)
        ones1 = sb("ones1", [1, 128], F32R)
        ident_sb = sb("ident_sb", [128, 128])
        diag_tmp = sb("diag_tmp", [128, 128])
        vdiag = sb("vdiag", [128, 1])
        sqv = sb("sqv", [128, 1])
        lnu = sb("lnu", [128, 1])
        s3v = sb("s3v", [128, 1])
        t1v = sb("t1v", [128, 1])
        fixp = sb("fixp", [128, 1])
        deltap = sb("deltap", [128, 1])
        lnS_b = sb("lnS_b", [1, 1])
        s2_b = sb("s2_b", [128, 1])

        # --- PSUM: 6 main banks + ssq row bank + broadcast bank = 8 ---
        ps_main = [ps(f"ps_main{b}") for b in range(NPS)]
        ps_ssq = ps("ps_ssq")
        ps_bc = ps("ps_bc")

        # --- semaphores ---
        s_const = sem("s_const")
        s_wt = [sem(f"s_wt{b}") for b in range(NWT)]
        s_dmaout = sem("s_dmaout")
        s_outdone = sem("s_outdone")
        s_sq = sem("s_sq")
        s_ssqmm = sem("s_ssqmm")
        s_row = sem("s_row")
        s_bcmm = sem("s_bcmm")
        s_bcev = sem("s_bcev")
        s_mmtile = sem("s_mmtile")
        s_evtile = sem("s_evtile")
        s_vg = sem("s_vg")
        s_sfix = sem("s_sfix")
        s_vfix = sem("s_vfix")
        s_esq = sem("s_esq")
        s_essqmm = sem("s_essqmm")
        s_erow = sem("s_erow")
        s_ebcmm = sem("s_ebcmm")
        s_ebcp = sem("s_ebcp")
        s_etn = sem("s_etn")

        with nc.Block() as block:

            @block.gpsimd
            def _(g):
                g.memset(lnS_b[:], float(np.log(S)))
                g.memset(s2_b[:], float(S * S))
                g.dma_start(out=ones128[:], in_=ones_ext[:].rearrange("(p o) -> p o", o=1)).then_inc(s_const, 16)
                g.dma_start(out=ones1[:], in_=ones_ext[:].rearrange("(o p) -> o p", o=1)).then_inc(s_const, 16)
                g.dma_start(out=ident_sb[:], in_=ident_ext[:]).then_inc(s_const, 16)
                g.dma_start(out=eT_sb[:], in_=eT_ext[:].rearrange("(ko p) n -> p ko n", p=128)).then_inc(s_const, 16)
                for c in range(min(NWT, NCHUNK)):
                    g.dma_start(
                        out=wt[c % NWT][:],
                        in_=wT_ext[:, c * F:(c + 1) * F].rearrange("(ko p) f -> p ko f", p=128),
                    ).then_inc(s_wt[c % NWT], 16)
                for c in range(NCHUNK):
                    cp = c + NWT
                    if cp < NCHUNK:
                        # buffer cp%NWT freed once main(cp-NWT) is done
                        g.wait_ge(s_mmtile, 8 * (cp - NWT + 1))
                        g.dma_start(
                            out=wt[cp % NWT][:],
                            in_=wT_ext[:, cp * F:(cp + 1) * F].rearrange("(ko p) f -> p ko f", p=128),
                        ).then_inc(s_wt[cp % NWT], 16)
                    g.wait_ge(s_evtile, 8 * (c + 1))
                    if c == 0:
                        g.wait_ge(s_vfix, 1)
                    for nb in range(NB):
                        t = c * NB + nb
                        g.dma_start(
                            out=out_ext[nb * 128:(nb + 1) * 128, c * F:(c + 1) * F],
                            in_=out_sb[t % NOUT][:],
                        ).then_inc(s_dmaout, 16)
                    # all out-DMAs issued so far == 8*(c+1): wait for full drain
                    g.wait_ge(s_dmaout, 16 * 8 * (c + 1))
                    g.engine_nop().then_inc(s_outdone, 1)

            @block.scalar
            def _(s):
                # e-prep
                s.wait_ge(s_const, 64)
                for k in range(KD):
                    ins = s.activation(esq[:, k, :], eT_sb[:, k, :], AF.Square)
                ins.then_inc(s_esq, 1)
                s.wait_ge(s_essqmm, 1)
                for h in range(2):
                    src = ps_main[h][0:1, :]
                    s.activation(lnw_tmp[:], src, AF.Ln)
                    ins = s.activation(rsqe_row[0:1, h * F:(h + 1) * F], lnw_tmp[:], AF.Exp, scale=-0.5, bias=lnS_b[:])
                ins.then_inc(s_erow, 1)
                s.wait_ge(s_ebcmm, 1)
                s.activation(rsqe_bc[:, 0:F], ps_main[2][:], AF.Copy)
                s.activation(rsqe_bc[:, F:N], ps_main[3][:], AF.Copy).then_inc(s_ebcp, 1)
                # chunk loop
                for c in range(NCHUNK):
                    s.wait_ge(s_wt[c % NWT], 16 * (c // NWT + 1))
                    if c >= 2:
                        s.wait_ge(s_ssqmm, c - 1)  # wsq[c%2] freed by ssq(c-2)
                    for k in range(KD):
                        ins = s.activation(wsq[c % 2][:, k, :], wt[c % NWT][:, k, :].bitcast(F32), AF.Square)
                    ins.then_inc(s_sq, 1)
                    s.wait_ge(s_ssqmm, c + 1)
                    s.activation(lnw_tmp[:], ps_ssq[0:1, :], AF.Ln)
                    s.activation(rsqw_row[c % 2][:], lnw_tmp[:], AF.Exp, scale=-0.5).then_inc(s_row, 1)
                    if c >= 1:
                        s.wait_ge(s_bcmm, c)
                        s.activation(rsqw_bc[(c - 1) % 2][:], ps_bc[:], AF.Copy).then_inc(s_bcev, 1)
                    if c == 1:
                        # margin fix math on the gathered diagonal [128,1]
                        s.wait_ge(s_vg, 1)
                        s.activation(sqv[:], vdiag[:], AF.Square)
                        s.drain()
                        s.activation(lnu[:], sqv[:], AF.Ln, scale=-1.0, bias=s2_b[:])
                        s.drain()
                        s.activation(s3v[:], lnu[:], AF.Exp, scale=0.5)
                        s.activation(t1v[:], vdiag[:], AF.Copy, scale=COSM)
                        s.drain().then_inc(s_sfix, 1)
                # tail: evict last broadcast
                s.wait_ge(s_bcmm, NCHUNK)
                s.activation(rsqw_bc[(NCHUNK - 1) % 2][:], ps_bc[:], AF.Copy).then_inc(s_bcev, 1)

            @block.tensor
            def _(t):
                # e-prep: ssq_e rows then rsqe broadcast
                t.wait_ge(s_esq, 1)
                for h in range(2):
                    for k in range(KD):
                        ins = t.matmul(
                            ps_main[h][0:1, :], lhsT=ones128[:],
                            rhs=esq[:, k, h * F:(h + 1) * F],
                            start=(k == 0), stop=(k == KD - 1),
                        )
                ins.then_inc(s_essqmm, 1)
                t.wait_ge(s_erow, 1)
                for h in range(2):
                    ins = t.matmul(ps_main[2 + h][:], lhsT=ones1[:],
                             rhs=rsqe_row[0:1, h * F:(h + 1) * F], start=True, stop=True)
                ins.then_inc(s_ebcmm, 1)
                # wait for eTn + eprep psum drains before main work
                t.wait_ge(s_etn, 1)
                t.wait_ge(s_ebcp, 1)
                for B in range(NCHUNK + 1):
                    if B <= NCHUNK - 1:
                        # ssq for chunk B
                        t.wait_ge(s_sq, B + 1)
                        if B >= 1:
                            t.wait_ge(s_row, B)  # ps_ssq freed by row-chain(B-1)
                        for k in range(KD):
                            ins = t.matmul(ps_ssq[0:1, :], lhsT=ones128[:], rhs=wsq[B % 2][:, k, :],
                                     start=(k == 0), stop=(k == KD - 1))
                        ins.then_inc(s_ssqmm, 1)
                    if B >= 1:
                        A = B - 1
                        # broadcast rsqw(A) to all partitions
                        t.wait_ge(s_row, A + 1)
                        if A >= 1:
                            t.wait_ge(s_bcev, A)  # ps_bc freed by bcevict(A-1)
                        t.matmul(ps_bc[:], lhsT=ones1[:], rhs=rsqw_row[A % 2][:], start=True, stop=True).then_inc(s_bcmm, 1)
                        # main matmuls for chunk A
                        for nb in range(NB):
                            tt = A * NB + nb
                            if tt >= NPS:
                                t.wait_ge(s_evtile, tt - NPS + 1)
                            for k in range(KD):
                                mm = t.matmul(
                                    ps_main[tt % NPS][:],
                                    lhsT=eTn[:, k, nb * 128:(nb + 1) * 128],
                                    rhs=wt[A % NWT][:, k, :],
                                    start=(k == 0), stop=(k == KD - 1),
                                )
                            mm.then_inc(s_mmtile, 1)

            @block.vector
            def _(v):
                # e-prep: eTn = eT * rsqe_bc  (f32r)
                v.wait_ge(s_const, 64)
                v.wait_ge(s_ebcp, 1)
                for k in range(KD):
                    v.tensor_tensor(out=eTn[:, k, :], in0=eT_sb[:, k, :], in1=rsqe_bc[:], op=OP.mult)
                v.engine_nop().then_inc(s_etn, 1)
                for c in range(NCHUNK):
                    for nb in range(NB):
                        tt = c * NB + nb
                        v.wait_ge(s_mmtile, tt + 1)
                        if nb == 0:
                            v.wait_ge(s_bcev, c + 1)
                        if tt >= NOUT:
                            v.wait_ge(s_outdone, c - 2)
                        v.tensor_tensor(out=out_sb[tt % NOUT][:], in0=ps_main[tt % NPS][:],
                                        in1=rsqw_bc[c % 2][:], op=OP.mult)
                        v.engine_nop().then_inc(s_evtile, 1)
                        if c == 0 and nb == 0:
                            # extract the label diagonal of the first tile
                            v.tensor_tensor(out=diag_tmp[:], in0=out_sb[0][:, 0:128],
                                            in1=ident_sb[:], op=OP.mult)
                            v.drain()
                            v.tensor_reduce(vdiag[:], diag_tmp[:],
                                            mybir.AxisListType.X, OP.add)
                            v.drain().then_inc(s_vg, 1)
                    if c == 1:
                        # apply margin on the diagonal of tile (0,0)
                        v.wait_ge(s_sfix, 1)
                        v.scalar_tensor_tensor(fixp[:], s3v[:], -SINM, t1v[:], OP.mult, OP.add)
                        v.drain()
                        v.tensor_tensor(out=deltap[:], in0=fixp[:], in1=vdiag[:], op=OP.subtract)
                        v.drain()
                        v.scalar_tensor_tensor(out_sb[0][:, 0:128], ident_sb[:], deltap[:],
                                               out_sb[0][:, 0:128], OP.mult, OP.add)
                        v.drain().then_inc(s_vfix, 1)

    return nc


_GRAPH = None


def _get_graph():
    global _GRAPH
    if _GRAPH is None:
        _GRAPH = build_graph()
    return _GRAPH


def _host_prepare(embeddings, weight, labels):
    """Row/class permutations putting each core's labels on the (0,0) diagonal."""
    labels = np.asarray(labels).astype(np.int64)
    e = np.asarray(embeddings, dtype=np.float32)
    w = np.asarray(weight, dtype=np.float32)

    # fix instance i (row i, class labels[i]) goes to core i//128, column i%128
    first_seen = {}
    extras = []  # (core, col, row, cls) for duplicate label classes
    primary_col = {}  # cls -> (core, col)
    for i in range(N):
        l = int(labels[i])
        m, p = i // 128, i % 128
        if l not in first_seen:
            first_seen[l] = (m, p)
            primary_col[l] = (m, p)
        else:
            extras.append((m, p, i, l))

    labeled = np.zeros(C, dtype=bool)
    labeled[labels] = True
    unlab = np.nonzero(~labeled)[0]

    # column map per core: -1 = padding column
    colmaps = np.full((NCORES, CS), -1, dtype=np.int64)
    for i in range(N):
        colmaps[i // 128, i % 128] = labels[i]
    fill_slots = NCORES * (CS - 128)
    fill = np.full(fill_slots, -1, dtype=np.int64)
    fill[: unlab.size] = unlab
    fill = fill.reshape(NCORES, CS - 128)
    colmaps[:, 128:] = fill

    # bulk-assign validity: skip pad and non-primary duplicate columns
    valid_bulk = colmaps >= 0
    for (m, p, i, l) in extras:
        valid_bulk[m, p] = False

    wTfull = w.T  # [512, 100000] view
    in_maps = []
    row_perms = []
    ones = np.ones(128, dtype=np.float32)
    ident = np.eye(128, dtype=np.float32)
    for m in range(NCORES):
        cm = colmaps[m]
        wt = np.zeros((D, CS), dtype=np.float32)
        vmask = cm >= 0
        wt[:, vmask] = wTfull[:, cm[vmask]]
        wt[0, ~vmask] = 1.0
        rows = np.concatenate([
            np.arange(m * 128, (m + 1) * 128),
            np.delete(np.arange(N), np.s_[m * 128:(m + 1) * 128]),
        ])
        row_perms.append(rows)
        eT = np.ascontiguousarray(e[rows].T)
        in_maps.append({
            "eT": eT,
            "wT": np.ascontiguousarray(wt),
            "ones": ones,
            "ident": ident,
        })
    return in_maps, row_perms, colmaps, valid_bulk, extras


def _assemble(results, row_perms, colmaps, valid_bulk, extras):
    out = np.empty((N, C), dtype=np.float32)
    slabs = []
    for m in range(NCORES):
        slab = results[m]["out"]
        unperm = np.empty_like(slab)
        unperm[row_perms[m]] = slab
        slabs.append(unperm)
        vb = valid_bulk[m]
        out[:, colmaps[m][vb]] = unperm[:, vb]
    for (m, p, i, l) in extras:
        out[i, l] = slabs[m][i, p]
    return out


def kernel(embeddings, weight, labels, _trace=False):
    nc = _get_graph()
    in_maps, row_perms, colmaps, valid_bulk, extras = _host_prepare(
        embeddings, weight, labels
    )
    res = run_bass_kernel_spmd(nc, in_maps, core_ids=list(range(NCORES)), trace=_trace)
    out = _assemble(res.results, row_perms, colmaps, valid_bulk, extras)
    if _trace:
        return out, res
    return out


# revision 7
# speedup vs baseline: 1.0111x; 1.0111x over previous
"""ArcFace logits on 8 Trainium2 NeuronCores (Bass, raw engine streams).

out[n, c] = S * cos(theta_nc + M * [c == labels[n]]),  cos from L2-normalized
embeddings [1024, 512] x weight [100000, 512].

Strategy: model-parallel over the class dim (partial-FC).  Classes are
padded/permuted on the host so that every core gets 12800 columns and its
128 label hits land on the diagonal of the first 128x128 output block.
That makes the compiled graph identical on all 8 cores and fully
label-independent: the margin fix is a cheap diagonal extract/rewrite with
an identity mask.  The host only moves data (transpose / permute / gather),
all FLOPs (normalization, matmul, margin trig) run on device.

Matmuls run in float32r (full-rate fp32, ~1.5e-4 rel err).  1/sqrt uses the
Ln/Exp activation tables (one table set, no reloads).
"""

import math

import numpy as np

import concourse.bass as bass
import concourse.mybir as mybir
from concourse.bass_utils import run_bass_kernel_spmd

AF = mybir.ActivationFunctionType
OP = mybir.AluOpType
F32 = mybir.dt.float32
F32R = mybir.dt.float32r
BF16 = mybir.dt.bfloat16

S = 30.0
MARGIN = 0.5
N, D, C = 1024, 512, 100000

NCORES = 8
CS = 12800            # classes per core (padded: 8 * 12800 = 102400)
CPAD = NCORES * CS
F = 512               # matmul free dim / class chunk width
NCHUNK = CS // F      # 25
KD = D // 128         # 4 contraction sub-tiles
NB = N // 128         # 8 row blocks
NTILES = NCHUNK * NB  # 200 output tiles per core
NPS = 6               # main PSUM bank rotation
NWT = 6               # wT chunk buffers
NOUT = 24             # out_sb rotation (3 chunks)

COSM = float(math.cos(MARGIN))
SINM = float(math.sin(MARGIN))


def build_graph():
    nc = bass.Bass(target_bir_lowering=False)

    eT_ext = nc.declare_dram_parameter("eT", [D, N], F32, isOutput=False)
    wT_ext = nc.declare_dram_parameter("wT", [D, CS], F32, isOutput=False)
    ones_ext = nc.declare_dram_parameter("ones", [128], F32, isOutput=False)
    ident_ext = nc.declare_dram_parameter("ident", [128, 128], F32, isOutput=False)
    out_ext = nc.declare_dram_parameter("out", [N, CS], F32, isOutput=True)

    import contextlib

    ctx = contextlib.ExitStack()
    sb = lambda name, shape, dt=F32: ctx.enter_context(nc.sbuf_tensor(name, shape, dt))
    ps = lambda name: ctx.enter_context(nc.psum_tensor(name, [128, F], F32))
    sem = lambda name: ctx.enter_context(nc.semaphore(name))

    with ctx:
        # --- SBUF ---
        eT_sb = sb("eT_sb", [128, KD, N])            # raw e^T
        esq = sb("esq", [128, KD, N], F32R)          # e^2 (rounded)
        eTn = sb("eTn", [128, KD, N], BF16)          # S * e / ||e||
        rsqe_row = sb("rsqe_row", [1, N], F32R)
        rsqe_bc = sb("rsqe_bc", [128, N])
        wt = [sb(f"wt{b}", [128, KD, F], BF16) for b in range(NWT)]
        wsq = [sb(f"wsq{b}", [128, KD, F], F32R) for b in range(3)]
        lnw_tmp = sb("lnw_tmp", [1, F])
        rsqw_row = [sb(f"rsqw_row{b}", [1, F], F32R) for b in range(2)]
        rsqw_bc = [sb(f"rsqw_bc{b}", [128, F]) for b in range(2)]
        out_sb = [sb(f"out_sb{b}", [128, F]) for b in range(NOUT)]
        ones128 = sb("ones128", [128, 1], F32R)
        ones1 = sb("ones1", [1, 128], F32R)
        ident_sb = sb("ident_sb", [128, 128])
        diag_tmp = sb("diag_tmp", [128, 128])
        vdiag = sb("vdiag", [128, 1])
        sqv = sb("sqv", [128, 1])
        lnu = sb("lnu", [128, 1])
        s3v = sb("s3v", [128, 1])
        t1v = sb("t1v", [128, 1])
        fixp = sb("fixp", [128, 1])
        deltap = sb("deltap", [128, 1])
        lnS_b = sb("lnS_b", [1, 1])
        s2_b = sb("s2_b", [128, 1])

        # --- PSUM: 6 main banks + ssq row bank + broadcast bank = 8 ---
        ps_main = [ps(f"ps_main{b}") for b in range(NPS)]
        ps_ssq = ps("ps_ssq")
        ps_bc = ps("ps_bc")

        # --- semaphores ---
        s_const = sem("s_const")
        s_wt = [sem(f"s_wt{b}") for b in range(NWT)]
        s_dmaout = sem("s_dmaout")
        s_outdone = sem("s_outdone")
        s_sq = sem("s_sq")
        s_ssqmm = sem("s_ssqmm")
        s_row = sem("s_row")
        s_bcmm = sem("s_bcmm")
        s_bcev = sem("s_bcev")
        s_mmtile = sem("s_mmtile")
        s_evtile = sem("s_evtile")
        s_vg = sem("s_vg")
        s_sfix = sem("s_sfix")
        s_vfix = sem("s_vfix")
        s_esq = sem("s_esq")
        s_essqmm = sem("s_essqmm")
        s_erow = sem("s_erow")
        s_ebcmm = sem("s_ebcmm")
        s_ebcp = sem("s_ebcp")
        s_etn = sem("s_etn")

        with nc.Block() as block:

            @block.gpsimd
            def _(g):
                g.memset(lnS_b[:], float(np.log(S)))
                g.memset(s2_b[:], float(S * S))
                g.dma_start(out=ones128[:], in_=ones_ext[:].rearrange("(p o) -> p o", o=1)).then_inc(s_const, 16)
                g.dma_start(out=ones1[:], in_=ones_ext[:].rearrange("(o p) -> o p", o=1)).then_inc(s_const, 16)
                g.dma_start(out=ident_sb[:], in_=ident_ext[:]).then_inc(s_const, 16)
                g.dma_start(out=eT_sb[:], in_=eT_ext[:].rearrange("(ko p) n -> p ko n", p=128)).then_inc(s_const, 16)
                for c in range(min(NWT, NCHUNK)):
                    g.dma_start(
                        out=wt[c % NWT][:],
                        in_=wT_ext[:, c * F:(c + 1) * F].rearrange("(ko p) f -> p ko f", p=128),
                    ).then_inc(s_wt[c % NWT], 16)
                for c in range(NCHUNK):
                    cp = c + NWT
                    if cp < NCHUNK:
                        # buffer cp%NWT freed once main(cp-NWT) is done
                        g.wait_ge(s_mmtile, 8 * (cp - NWT + 1))
                        g.dma_start(
                            out=wt[cp % NWT][:],
                            in_=wT_ext[:, cp * F:(cp + 1) * F].rearrange("(ko p) f -> p ko f", p=128),
                        ).then_inc(s_wt[cp % NWT], 16)
                    g.wait_ge(s_evtile, 8 * (c + 1))
                    if c == 0:
                        g.wait_ge(s_vfix, 1)
                    for nb in range(NB):
                        t = c * NB + nb
                        g.dma_start(
                            out=out_ext[nb * 128:(nb + 1) * 128, c * F:(c + 1) * F],
                            in_=out_sb[t % NOUT][:],
                        ).then_inc(s_dmaout, 16)
                    # all out-DMAs issued so far == 8*(c+1): wait for full drain
                    g.wait_ge(s_dmaout, 16 * 8 * (c + 1))
                    g.engine_nop().then_inc(s_outdone, 1)

            @block.scalar
            def _(s):
                def do_squares(x):
                    s.wait_ge(s_wt[x % NWT], 16 * (x // NWT + 1))
                    if x >= 3:
                        s.wait_ge(s_ssqmm, x - 2)  # wsq[x%3] freed by ssq(x-3)
                    for k in range(KD):
                        ins = s.activation(wsq[x % 3][:, k, :], wt[x % NWT][:, k, :], AF.Square)
                    ins.then_inc(s_sq, 1)

                # wT squares for chunks 0/1 as early as possible
                do_squares(0)
                do_squares(1)
                # e-prep
                s.wait_ge(s_const, 64)
                for k in range(KD):
                    ins = s.activation(esq[:, k, :], eT_sb[:, k, :], AF.Square)
                ins.then_inc(s_esq, 1)
                s.wait_ge(s_essqmm, 1)
                for h in range(2):
                    src = ps_main[h][0:1, :]
                    s.activation(lnw_tmp[:], src, AF.Ln)
                    ins = s.activation(rsqe_row[0:1, h * F:(h + 1) * F], lnw_tmp[:], AF.Exp, scale=-0.5, bias=lnS_b[:])
                ins.then_inc(s_erow, 1)
                s.wait_ge(s_ebcmm, 1)
                s.activation(rsqe_bc[:, 0:F], ps_main[2][:], AF.Copy)
                s.activation(rsqe_bc[:, F:N], ps_main[3][:], AF.Copy).then_inc(s_ebcp, 1)
                # chunk loop: bcast-evict first (unblocks VectorE), then
                # squares two chunks ahead, then the rsqw row chain
                for c in range(NCHUNK):
                    if c >= 1:
                        s.wait_ge(s_bcmm, c)
                        s.activation(rsqw_bc[(c - 1) % 2][:], ps_bc[:], AF.Copy).then_inc(s_bcev, 1)
                    if c + 2 <= NCHUNK - 1:
                        do_squares(c + 2)
                    s.wait_ge(s_ssqmm, c + 1)
                    s.activation(lnw_tmp[:], ps_ssq[0:1, :], AF.Ln)
                    s.activation(rsqw_row[c % 2][:], lnw_tmp[:], AF.Exp, scale=-0.5).then_inc(s_row, 1)
                    if c == 1:
                        # margin fix math on the gathered diagonal [128,1]
                        s.wait_ge(s_vg, 1)
                        s.activation(sqv[:], vdiag[:], AF.Square)
                        s.drain()
                        s.activation(lnu[:], sqv[:], AF.Ln, scale=-1.0, bias=s2_b[:])
                        s.drain()
                        s.activation(s3v[:], lnu[:], AF.Exp, scale=0.5)
                        s.activation(t1v[:], vdiag[:], AF.Copy, scale=COSM)
                        s.drain().then_inc(s_sfix, 1)
                # tail: evict last broadcast
                s.wait_ge(s_bcmm, NCHUNK)
                s.activation(rsqw_bc[(NCHUNK - 1) % 2][:], ps_bc[:], AF.Copy).then_inc(s_bcev, 1)

            @block.tensor
            def _(t):
                # chunk-0 ssq as soon as its squares land (keeps PE warm early)
                t.wait_ge(s_sq, 1)
                for k in range(KD):
                    ins = t.matmul(ps_ssq[0:1, :], lhsT=ones128[:], rhs=wsq[0][:, k, :],
                             start=(k == 0), stop=(k == KD - 1))
                ins.then_inc(s_ssqmm, 1)
                # e-prep: ssq_e rows then rsqe broadcast
                t.wait_ge(s_esq, 1)
                for h in range(2):
                    for k in range(KD):
                        ins = t.matmul(
                            ps_main[h][0:1, :], lhsT=ones128[:],
                            rhs=esq[:, k, h * F:(h + 1) * F],
                            start=(k == 0), stop=(k == KD - 1),
                        )
                ins.then_inc(s_essqmm, 1)
                t.wait_ge(s_erow, 1)
                for h in range(2):
                    ins = t.matmul(ps_main[2 + h][:], lhsT=ones1[:],
                             rhs=rsqe_row[0:1, h * F:(h + 1) * F], start=True, stop=True)
                ins.then_inc(s_ebcmm, 1)
                # wait for eTn + eprep psum drains before main work
                t.wait_ge(s_etn, 1)
                t.wait_ge(s_ebcp, 1)
                for B in range(1, NCHUNK + 1):
                    if B <= NCHUNK - 1:
                        # ssq for chunk B
                        t.wait_ge(s_sq, B + 1)
                        t.wait_ge(s_row, B)  # ps_ssq freed by row-chain(B-1)
                        for k in range(KD):
                            ins = t.matmul(ps_ssq[0:1, :], lhsT=ones128[:], rhs=wsq[B % 3][:, k, :],
                                     start=(k == 0), stop=(k == KD - 1))
                        ins.then_inc(s_ssqmm, 1)
                    if B >= 1:
                        A = B - 1
                        # broadcast rsqw(A) to all partitions
                        t.wait_ge(s_row, A + 1)
                        if A >= 1:
                            t.wait_ge(s_bcev, A)  # ps_bc freed by bcevict(A-1)
                        t.matmul(ps_bc[:], lhsT=ones1[:], rhs=rsqw_row[A % 2][:], start=True, stop=True).then_inc(s_bcmm, 1)
                        # main matmuls for chunk A
                        for nb in range(NB):
                            tt = A * NB + nb
                            if tt >= NPS:
                                t.wait_ge(s_evtile, tt - NPS + 1)
                            for k in range(KD):
                                mm = t.matmul(
                                    ps_main[tt % NPS][:],
                                    lhsT=eTn[:, k, nb * 128:(nb + 1) * 128],
                                    rhs=wt[A % NWT][:, k, :],
                                    start=(k == 0), stop=(k == KD - 1),
                                )
                            mm.then_inc(s_mmtile, 1)

            @block.vector
            def _(v):
                # e-prep: eTn = eT * rsqe_bc  (f32r)
                v.wait_ge(s_const, 64)
                v.wait_ge(s_ebcp, 1)
                for k in range(KD):
                    v.tensor_tensor(out=eTn[:, k, :], in0=eT_sb[:, k, :], in1=rsqe_bc[:], op=OP.mult)
                v.engine_nop().then_inc(s_etn, 1)
                for c in range(NCHUNK):
                    for nb in range(NB):
                        tt = c * NB + nb
                        v.wait_ge(s_mmtile, tt + 1)
                        if nb == 0:
                            v.wait_ge(s_bcev, c + 1)
                        if tt >= NOUT:
                            v.wait_ge(s_outdone, c - 2)
                        v.tensor_tensor(out=out_sb[tt % NOUT][:], in0=ps_main[tt % NPS][:],
                                        in1=rsqw_bc[c % 2][:], op=OP.mult)
                        v.engine_nop().then_inc(s_evtile, 1)
                        if c == 0 and nb == 0:
                            # extract the label diagonal of the first tile
                            v.tensor_tensor(out=diag_tmp[:], in0=out_sb[0][:, 0:128],
                                            in1=ident_sb[:], op=OP.mult)
                            v.drain()
                            v.tensor_reduce(vdiag[:], diag_tmp[:],
                                            mybir.AxisListType.X, OP.add)
                            v.drain().then_inc(s_vg, 1)
                    if c == 1:
                        # apply margin on the diagonal of tile (0,0)
                        v.wait_ge(s_sfix, 1)
                        v.scalar_tensor_tensor(fixp[:], s3v[:], -SINM, t1v[:], OP.mult, OP.add)
                        v.drain()
                        v.tensor_tensor(out=deltap[:], in0=fixp[:], in1=vdiag[:], op=OP.subtract)
                        v.drain()
                        v.scalar_tensor_tensor(out_sb[0][:, 0:128], ident_sb[:], deltap[:],
                                               out_sb[0][:, 0:128], OP.mult, OP.add)
                        v.drain().then_inc(s_vfix, 1)

    return nc


_GRAPH = None


def _get_graph():
    global _GRAPH
    if _GRAPH is None:
        _GRAPH = build_graph()
    return _GRAPH


def _host_prepare(embeddings, weight, labels):
    """Row/class permutations putting each core's labels on the (0,0) diagonal."""
    labels = np.asarray(labels).astype(np.int64)
    e = np.asarray(embeddings, dtype=np.float32)
    w = np.asarray(weight, dtype=np.float32)

    # fix instance i (row i, class labels[i]) goes to core i//128, column i%128
    first_seen = {}
    extras = []  # (core, col, row, cls) for duplicate label classes
    primary_col = {}  # cls -> (core, col)
    for i in range(N):
        l = int(labels[i])
        m, p = i // 128, i % 128
        if l not in first_seen:
            first_seen[l] = (m, p)
            primary_col[l] = (m, p)
        else:
            extras.append((m, p, i, l))

    labeled = np.zeros(C, dtype=bool)
    labeled[labels] = True
    unlab = np.nonzero(~labeled)[0]

    # column map per core: -1 = padding column
    colmaps = np.full((NCORES, CS), -1, dtype=np.int64)
    for i in range(N):
        colmaps[i // 128, i % 128] = labels[i]
    fill_slots = NCORES * (CS - 128)
    fill = np.full(fill_slots, -1, dtype=np.int64)
    fill[: unlab.size] = unlab
    fill = fill.reshape(NCORES, CS - 128)
    colmaps[:, 128:] = fill

    # bulk-assign validity: skip pad and non-primary duplicate columns
    valid_bulk = colmaps >= 0
    for (m, p, i, l) in extras:
        valid_bulk[m, p] = False

    wTfull = w.T  # [512, 100000] view
    in_maps = []
    row_perms = []
    ones = np.ones(128, dtype=np.float32)
    ident = np.eye(128, dtype=np.float32)
    for m in range(NCORES):
        cm = colmaps[m]
        wt = np.zeros((D, CS), dtype=np.float32)
        vmask = cm >= 0
        wt[:, vmask] = wTfull[:, cm[vmask]]
        wt[0, ~vmask] = 1.0
        rows = np.concatenate([
            np.arange(m * 128, (m + 1) * 128),
            np.delete(np.arange(N), np.s_[m * 128:(m + 1) * 128]),
        ])
        row_perms.append(rows)
        eT = np.ascontiguousarray(e[rows].T)
        in_maps.append({
            "eT": eT,
            "wT": np.ascontiguousarray(wt),
            "ones": ones,
            "ident": ident,
        })
    return in_maps, row_perms, colmaps, valid_bulk, extras


def _assemble(results, row_perms, colmaps, valid_bulk, extras):
    out = np.empty((N, C), dtype=np.float32)
    slabs = []
    for m in range(NCORES):
        slab = results[m]["out"]
        unperm = np.empty_like(slab)
        unperm[row_perms[m]] = slab
        slabs.append(unperm)
        vb = valid_bulk[m]
        out[:, colmaps[m][vb]] = unperm[:, vb]
    for (m, p, i, l) in extras:
        out[i, l] = slabs[m][i, p]
    return out


def kernel(embeddings, weight, labels, _trace=False):
    nc = _get_graph()
    in_maps, row_perms, colmaps, valid_bulk, extras = _host_prepare(
        embeddings, weight, labels
    )
    res = run_bass_kernel_spmd(nc, in_maps, core_ids=list(range(NCORES)), trace=_trace)
    out = _assemble(res.results, row_perms, colmaps, valid_bulk, extras)
    if _trace:
        return out, res
    return out


# revision 8
# speedup vs baseline: 1.1693x; 1.1565x over previous
"""ArcFace logits on 8 Trainium2 NeuronCores (Bass, raw engine streams).

out[n, c] = S * cos(theta_nc + M * [c == labels[n]]),  cos from L2-normalized
embeddings [1024, 512] x weight [100000, 512].

Strategy: model-parallel over the class dim (partial-FC).  Classes are
padded/permuted on the host so that every core gets 12800 columns and its
128 label hits land on the diagonal of the first 128x128 output block.
That makes the compiled graph identical on all 8 cores and fully
label-independent: the margin fix is a cheap diagonal extract/rewrite with
an identity mask.  The host only moves data (transpose / permute / gather),
all FLOPs (normalization, matmul, margin trig) run on device.

Matmuls run in float32r (full-rate fp32, ~1.5e-4 rel err).  1/sqrt uses the
Ln/Exp activation tables (one table set, no reloads).
"""

import math

import numpy as np

import concourse.bass as bass
import concourse.mybir as mybir
from concourse.bass_utils import run_bass_kernel_spmd

AF = mybir.ActivationFunctionType
OP = mybir.AluOpType
F32 = mybir.dt.float32
F32R = mybir.dt.float32r
BF16 = mybir.dt.bfloat16

S = 30.0
MARGIN = 0.5
N, D, C = 1024, 512, 100000

NCORES = 8
CS = 12800            # classes per core (padded: 8 * 12800 = 102400)
CPAD = NCORES * CS
F = 512               # matmul free dim / class chunk width
NCHUNK = CS // F      # 25
KD = D // 128         # 4 contraction sub-tiles
NB = N // 128         # 8 row blocks
NTILES = NCHUNK * NB  # 200 output tiles per core
NPS = 6               # main PSUM bank rotation
NWT = 6               # wT chunk buffers
NOUT = 32             # out_sb rotation (4 chunks)

COSM = float(math.cos(MARGIN))
SINM = float(math.sin(MARGIN))


def build_graph():
    nc = bass.Bass(target_bir_lowering=False)

    eT_ext = nc.declare_dram_parameter("eT", [D, N], F32, isOutput=False)
    wT_ext = nc.declare_dram_parameter("wT", [D, CS], F32, isOutput=False)
    ones_ext = nc.declare_dram_parameter("ones", [128], F32, isOutput=False)
    ident_ext = nc.declare_dram_parameter("ident", [128, 128], F32, isOutput=False)
    out_ext = nc.declare_dram_parameter("out", [N, CS], F32, isOutput=True)

    import contextlib

    ctx = contextlib.ExitStack()
    sb = lambda name, shape, dt=F32: ctx.enter_context(nc.sbuf_tensor(name, shape, dt))
    ps = lambda name: ctx.enter_context(nc.psum_tensor(name, [128, F], F32))
    sem = lambda name: ctx.enter_context(nc.semaphore(name))

    with ctx:
        # --- SBUF ---
        eT_sb = sb("eT_sb", [128, KD, N])            # raw e^T
        esq = sb("esq", [128, KD, N], F32R)          # e^2 (rounded)
        eTn = sb("eTn", [128, KD, N], BF16)          # S * e / ||e||
        rsqe_row = sb("rsqe_row", [1, N], F32R)
        rsqe_bc = sb("rsqe_bc", [128, N])
        wt = [sb(f"wt{b}", [128, KD, F], BF16) for b in range(NWT)]
        wsq = [sb(f"wsq{b}", [128, KD, F], F32R) for b in range(3)]
        lnw_tmp = sb("lnw_tmp", [1, F])
        rsqw_row = [sb(f"rsqw_row{b}", [1, F], F32R) for b in range(2)]
        rsqw_bc = [sb(f"rsqw_bc{b}", [128, F]) for b in range(2)]
        out_sb = [sb(f"out_sb{b}", [128, F]) for b in range(NOUT)]
        ones128 = sb("ones128", [128, 1], F32R)
        ones1 = sb("ones1", [1, 128], F32R)
        ident_sb = sb("ident_sb", [128, 128])
        diag_tmp = sb("diag_tmp", [128, 128])
        vdiag = sb("vdiag", [128, 1])
        sqv = sb("sqv", [128, 1])
        lnu = sb("lnu", [128, 1])
        s3v = sb("s3v", [128, 1])
        t1v = sb("t1v", [128, 1])
        fixp = sb("fixp", [128, 1])
        deltap = sb("deltap", [128, 1])
        lnS_b = sb("lnS_b", [1, 1])
        s2_b = sb("s2_b", [128, 1])

        # --- PSUM: 6 main banks + ssq row bank + broadcast bank = 8 ---
        ps_main = [ps(f"ps_main{b}") for b in range(NPS)]
        ps_ssq = ps("ps_ssq")
        ps_bc = ps("ps_bc")

        # --- semaphores ---
        s_const = sem("s_const")
        s_wt = [sem(f"s_wt{b}") for b in range(NWT)]
        s_dmaout = sem("s_dmaout")
        s_outdone = sem("s_outdone")
        s_sq = sem("s_sq")
        s_ssqmm = sem("s_ssqmm")
        s_row = sem("s_row")
        s_bcmm = sem("s_bcmm")
        s_bcev = sem("s_bcev")
        s_mmtile = sem("s_mmtile")
        s_evtile = sem("s_evtile")
        s_vg = sem("s_vg")
        s_sfix = sem("s_sfix")
        s_vfix = sem("s_vfix")
        s_esq = sem("s_esq")
        s_essqmm = sem("s_essqmm")
        s_erow = sem("s_erow")
        s_ebcmm = sem("s_ebcmm")
        s_ebcp = sem("s_ebcp")
        s_etn = sem("s_etn")

        with nc.Block() as block:

            @block.gpsimd
            def _(g):
                g.memset(lnS_b[:], float(np.log(S)))
                g.memset(s2_b[:], float(S * S))
                g.dma_start(out=ones128[:], in_=ones_ext[:].rearrange("(p o) -> p o", o=1)).then_inc(s_const, 16)
                g.dma_start(out=ones1[:], in_=ones_ext[:].rearrange("(o p) -> o p", o=1)).then_inc(s_const, 16)
                g.dma_start(out=ident_sb[:], in_=ident_ext[:]).then_inc(s_const, 16)
                g.dma_start(out=eT_sb[:], in_=eT_ext[:].rearrange("(ko p) n -> p ko n", p=128)).then_inc(s_const, 16)
                for c in range(min(NWT, NCHUNK)):
                    g.dma_start(
                        out=wt[c % NWT][:],
                        in_=wT_ext[:, c * F:(c + 1) * F].rearrange("(ko p) f -> p ko f", p=128),
                    ).then_inc(s_wt[c % NWT], 16)
                for c in range(NCHUNK):
                    cp = c + NWT
                    if cp < NCHUNK:
                        # buffer cp%NWT freed once main(cp-NWT) is done
                        g.wait_ge(s_mmtile, 8 * (cp - NWT + 1))
                        g.dma_start(
                            out=wt[cp % NWT][:],
                            in_=wT_ext[:, cp * F:(cp + 1) * F].rearrange("(ko p) f -> p ko f", p=128),
                        ).then_inc(s_wt[cp % NWT], 16)
                    if c == 0:
                        g.wait_ge(s_vfix, 1)
                    for nb in range(NB):
                        t = c * NB + nb
                        g.wait_ge(s_evtile, t + 1)
                        g.dma_start(
                            out=out_ext[nb * 128:(nb + 1) * 128, c * F:(c + 1) * F],
                            in_=out_sb[t % NOUT][:],
                        ).then_inc(s_dmaout, 16)
                # make sure all output DMAs have landed before the graph ends
                g.wait_ge(s_dmaout, 16 * 8 * NCHUNK)

            @block.scalar
            def _(s):
                def do_squares(x):
                    s.wait_ge(s_wt[x % NWT], 16 * (x // NWT + 1))
                    if x >= 3:
                        s.wait_ge(s_ssqmm, x - 2)  # wsq[x%3] freed by ssq(x-3)
                    for k in range(KD):
                        ins = s.activation(wsq[x % 3][:, k, :], wt[x % NWT][:, k, :], AF.Square)
                    ins.then_inc(s_sq, 1)

                # wT squares for chunks 0/1 as early as possible
                do_squares(0)
                do_squares(1)
                # e-prep
                s.wait_ge(s_const, 64)
                for k in range(KD):
                    ins = s.activation(esq[:, k, :], eT_sb[:, k, :], AF.Square)
                ins.then_inc(s_esq, 1)
                s.wait_ge(s_essqmm, 1)
                for h in range(2):
                    src = ps_main[h][0:1, :]
                    s.activation(lnw_tmp[:], src, AF.Ln)
                    ins = s.activation(rsqe_row[0:1, h * F:(h + 1) * F], lnw_tmp[:], AF.Exp, scale=-0.5, bias=lnS_b[:])
                ins.then_inc(s_erow, 1)
                s.wait_ge(s_ebcmm, 1)
                s.activation(rsqe_bc[:, 0:F], ps_main[2][:], AF.Copy)
                s.activation(rsqe_bc[:, F:N], ps_main[3][:], AF.Copy).then_inc(s_ebcp, 1)
                # chunk loop: bcast-evict first (unblocks VectorE), then
                # squares two chunks ahead, then the rsqw row chain
                for c in range(NCHUNK):
                    if c >= 1:
                        s.wait_ge(s_bcmm, c)
                        s.activation(rsqw_bc[(c - 1) % 2][:], ps_bc[:], AF.Copy).then_inc(s_bcev, 1)
                    if c + 2 <= NCHUNK - 1:
                        do_squares(c + 2)
                    s.wait_ge(s_ssqmm, c + 1)
                    s.activation(lnw_tmp[:], ps_ssq[0:1, :], AF.Ln)
                    s.activation(rsqw_row[c % 2][:], lnw_tmp[:], AF.Exp, scale=-0.5).then_inc(s_row, 1)
                    if c == 1:
                        # margin fix math on the gathered diagonal [128,1]
                        s.wait_ge(s_vg, 1)
                        s.activation(sqv[:], vdiag[:], AF.Square)
                        s.drain()
                        s.activation(lnu[:], sqv[:], AF.Ln, scale=-1.0, bias=s2_b[:])
                        s.drain()
                        s.activation(s3v[:], lnu[:], AF.Exp, scale=0.5)
                        s.activation(t1v[:], vdiag[:], AF.Copy, scale=COSM)
                        s.drain().then_inc(s_sfix, 1)
                # tail: evict last broadcast
                s.wait_ge(s_bcmm, NCHUNK)
                s.activation(rsqw_bc[(NCHUNK - 1) % 2][:], ps_bc[:], AF.Copy).then_inc(s_bcev, 1)

            @block.tensor
            def _(t):
                # chunk-0 ssq as soon as its squares land (keeps PE warm early)
                t.wait_ge(s_sq, 1)
                for k in range(KD):
                    ins = t.matmul(ps_ssq[0:1, :], lhsT=ones128[:], rhs=wsq[0][:, k, :],
                             start=(k == 0), stop=(k == KD - 1))
                ins.then_inc(s_ssqmm, 1)
                # e-prep: ssq_e rows then rsqe broadcast
                t.wait_ge(s_esq, 1)
                for h in range(2):
                    for k in range(KD):
                        ins = t.matmul(
                            ps_main[h][0:1, :], lhsT=ones128[:],
                            rhs=esq[:, k, h * F:(h + 1) * F],
                            start=(k == 0), stop=(k == KD - 1),
                        )
                ins.then_inc(s_essqmm, 1)
                t.wait_ge(s_erow, 1)
                for h in range(2):
                    ins = t.matmul(ps_main[2 + h][:], lhsT=ones1[:],
                             rhs=rsqe_row[0:1, h * F:(h + 1) * F], start=True, stop=True)
                ins.then_inc(s_ebcmm, 1)
                # wait for eTn + eprep psum drains before main work
                t.wait_ge(s_etn, 1)
                t.wait_ge(s_ebcp, 1)
                for B in range(1, NCHUNK + 1):
                    if B <= NCHUNK - 1:
                        # ssq for chunk B
                        t.wait_ge(s_sq, B + 1)
                        t.wait_ge(s_row, B)  # ps_ssq freed by row-chain(B-1)
                        for k in range(KD):
                            ins = t.matmul(ps_ssq[0:1, :], lhsT=ones128[:], rhs=wsq[B % 3][:, k, :],
                                     start=(k == 0), stop=(k == KD - 1))
                        ins.then_inc(s_ssqmm, 1)
                    if B >= 1:
                        A = B - 1
                        # broadcast rsqw(A) to all partitions
                        t.wait_ge(s_row, A + 1)
                        if A >= 1:
                            t.wait_ge(s_bcev, A)  # ps_bc freed by bcevict(A-1)
                        t.matmul(ps_bc[:], lhsT=ones1[:], rhs=rsqw_row[A % 2][:], start=True, stop=True).then_inc(s_bcmm, 1)
                        # main matmuls for chunk A
                        for nb in range(NB):
                            tt = A * NB + nb
                            if tt >= NPS:
                                t.wait_ge(s_evtile, tt - NPS + 1)
                            for k in range(KD):
                                mm = t.matmul(
                                    ps_main[tt % NPS][:],
                                    lhsT=eTn[:, k, nb * 128:(nb + 1) * 128],
                                    rhs=wt[A % NWT][:, k, :],
                                    start=(k == 0), stop=(k == KD - 1),
                                )
                            mm.then_inc(s_mmtile, 1)

            @block.vector
            def _(v):
                # e-prep: eTn = eT * rsqe_bc  (f32r)
                v.wait_ge(s_const, 64)
                v.wait_ge(s_ebcp, 1)
                for k in range(KD):
                    v.tensor_tensor(out=eTn[:, k, :], in0=eT_sb[:, k, :], in1=rsqe_bc[:], op=OP.mult)
                v.engine_nop().then_inc(s_etn, 1)
                for c in range(NCHUNK):
                    for nb in range(NB):
                        tt = c * NB + nb
                        v.wait_ge(s_mmtile, tt + 1)
                        if nb == 0:
                            v.wait_ge(s_bcev, c + 1)
                        if tt >= NOUT:
                            # buffer reused from chunk c-4; DMA completions are
                            # near-FIFO (every DMA sprays all 16 queues)
                            v.wait_ge(s_dmaout, 16 * (tt - NOUT + 8))
                        v.tensor_tensor(out=out_sb[tt % NOUT][:], in0=ps_main[tt % NPS][:],
                                        in1=rsqw_bc[c % 2][:], op=OP.mult)
                        v.engine_nop().then_inc(s_evtile, 1)
                        if c == 0 and nb == 0:
                            # extract the label diagonal of the first tile
                            v.tensor_tensor(out=diag_tmp[:], in0=out_sb[0][:, 0:128],
                                            in1=ident_sb[:], op=OP.mult)
                            v.drain()
                            v.tensor_reduce(vdiag[:], diag_tmp[:],
                                            mybir.AxisListType.X, OP.add)
                            v.drain().then_inc(s_vg, 1)
                    if c == 1:
                        # apply margin on the diagonal of tile (0,0)
                        v.wait_ge(s_sfix, 1)
                        v.scalar_tensor_tensor(fixp[:], s3v[:], -SINM, t1v[:], OP.mult, OP.add)
                        v.drain()
                        v.tensor_tensor(out=deltap[:], in0=fixp[:], in1=vdiag[:], op=OP.subtract)
                        v.drain()
                        v.scalar_tensor_tensor(out_sb[0][:, 0:128], ident_sb[:], deltap[:],
                                               out_sb[0][:, 0:128], OP.mult, OP.add)
                        v.drain().then_inc(s_vfix, 1)

    return nc


_GRAPH = None


def _get_graph():
    global _GRAPH
    if _GRAPH is None:
        _GRAPH = build_graph()
    return _GRAPH


def _host_prepare(embeddings, weight, labels):
    """Row/class permutations putting each core's labels on the (0,0) diagonal."""
    labels = np.asarray(labels).astype(np.int64)
    e = np.asarray(embeddings, dtype=np.float32)
    w = np.asarray(weight, dtype=np.float32)

    # fix instance i (row i, class labels[i]) goes to core i//128, column i%128
    first_seen = {}
    extras = []  # (core, col, row, cls) for duplicate label classes
    primary_col = {}  # cls -> (core, col)
    for i in range(N):
        l = int(labels[i])
        m, p = i // 128, i % 128
        if l not in first_seen:
            first_seen[l] = (m, p)
            primary_col[l] = (m, p)
        else:
            extras.append((m, p, i, l))

    labeled = np.zeros(C, dtype=bool)
    labeled[labels] = True
    unlab = np.nonzero(~labeled)[0]

    # column map per core: -1 = padding column
    colmaps = np.full((NCORES, CS), -1, dtype=np.int64)
    for i in range(N):
        colmaps[i // 128, i % 128] = labels[i]
    fill_slots = NCORES * (CS - 128)
    fill = np.full(fill_slots, -1, dtype=np.int64)
    fill[: unlab.size] = unlab
    fill = fill.reshape(NCORES, CS - 128)
    colmaps[:, 128:] = fill

    # bulk-assign validity: skip pad and non-primary duplicate columns
    valid_bulk = colmaps >= 0
    for (m, p, i, l) in extras:
        valid_bulk[m, p] = False

    wTfull = w.T  # [512, 100000] view
    in_maps = []
    row_perms = []
    ones = np.ones(128, dtype=np.float32)
    ident = np.eye(128, dtype=np.float32)
    for m in range(NCORES):
        cm = colmaps[m]
        wt = np.zeros((D, CS), dtype=np.float32)
        vmask = cm >= 0
        wt[:, vmask] = wTfull[:, cm[vmask]]
        wt[0, ~vmask] = 1.0
        rows = np.concatenate([
            np.arange(m * 128, (m + 1) * 128),
            np.delete(np.arange(N), np.s_[m * 128:(m + 1) * 128]),
        ])
        row_perms.append(rows)
        eT = np.ascontiguousarray(e[rows].T)
        in_maps.append({
            "eT": eT,
            "wT": np.ascontiguousarray(wt),
            "ones": ones,
            "ident": ident,
        })
    return in_maps, row_perms, colmaps, valid_bulk, extras


def _assemble(results, row_perms, colmaps, valid_bulk, extras):
    out = np.empty((N, C), dtype=np.float32)
    slabs = []
    for m in range(NCORES):
        slab = results[m]["out"]
        unperm = np.empty_like(slab)
        unperm[row_perms[m]] = slab
        slabs.append(unperm)
        vb = valid_bulk[m]
        out[:, colmaps[m][vb]] = unperm[:, vb]
    for (m, p, i, l) in extras:
        out[i, l] = slabs[m][i, p]
    return out


def kernel(embeddings, weight, labels, _trace=False):
    nc = _get_graph()
    in_maps, row_perms, colmaps, valid_bulk, extras = _host_prepare(
        embeddings, weight, labels
    )
    res = run_bass_kernel_spmd(nc, in_maps, core_ids=list(range(NCORES)), trace=_trace)
    out = _assemble(res.results, row_perms, colmaps, valid_bulk, extras)
    if _trace:
        return out, res
    return out


# revision 9
# speedup vs baseline: 1.1813x; 1.0102x over previous
"""ArcFace logits on 8 Trainium2 NeuronCores (Bass, raw engine streams).

out[n, c] = S * cos(theta_nc + M * [c == labels[n]]),  cos from L2-normalized
embeddings [1024, 512] x weight [100000, 512].

Strategy: model-parallel over the class dim (partial-FC).  Classes are
padded/permuted on the host so that every core gets 12800 columns and its
128 label hits land on the diagonal of the first 128x128 output block.
That makes the compiled graph identical on all 8 cores and fully
label-independent: the margin fix is a cheap diagonal extract/rewrite with
an identity mask.  The host only moves data (transpose / permute / gather),
all FLOPs (normalization, matmul, margin trig) run on device.

Matmuls run in float32r (full-rate fp32, ~1.5e-4 rel err).  1/sqrt uses the
Ln/Exp activation tables (one table set, no reloads).
"""

import math

import numpy as np

import concourse.bass as bass
import concourse.mybir as mybir
from concourse.bass_utils import run_bass_kernel_spmd

AF = mybir.ActivationFunctionType
OP = mybir.AluOpType
F32 = mybir.dt.float32
F32R = mybir.dt.float32r
BF16 = mybir.dt.bfloat16

S = 30.0
MARGIN = 0.5
N, D, C = 1024, 512, 100000

NCORES = 8
CS = 12800            # classes per core (padded: 8 * 12800 = 102400)
CPAD = NCORES * CS
F = 512               # matmul free dim / class chunk width
NCHUNK = CS // F      # 25
KD = D // 128         # 4 contraction sub-tiles
NB = N // 128         # 8 row blocks
NTILES = NCHUNK * NB  # 200 output tiles per core
NPS = 6               # main PSUM bank rotation
NWT = 6               # wT chunk buffers
NOUT = 32             # out_sb rotation (4 chunks)

COSM = float(math.cos(MARGIN))
SINM = float(math.sin(MARGIN))


def build_graph():
    nc = bass.Bass(target_bir_lowering=False)

    eT_ext = nc.declare_dram_parameter("eT", [D, N], F32, isOutput=False)
    wT_ext = nc.declare_dram_parameter("wT", [D, CS], F32, isOutput=False)
    ones_ext = nc.declare_dram_parameter("ones", [128], F32, isOutput=False)
    ident_ext = nc.declare_dram_parameter("ident", [128, 128], F32, isOutput=False)
    out_ext = nc.declare_dram_parameter("out", [N, CS], F32, isOutput=True)

    import contextlib

    ctx = contextlib.ExitStack()
    sb = lambda name, shape, dt=F32: ctx.enter_context(nc.sbuf_tensor(name, shape, dt))
    ps = lambda name: ctx.enter_context(nc.psum_tensor(name, [128, F], F32))
    sem = lambda name: ctx.enter_context(nc.semaphore(name))

    with ctx:
        # --- SBUF ---
        eT_sb = sb("eT_sb", [128, KD, N])            # raw e^T
        esq = sb("esq", [128, KD, N], F32R)          # e^2 (rounded)
        eTn = sb("eTn", [128, KD, N], BF16)          # S * e / ||e||
        rsqe_row = sb("rsqe_row", [1, N], F32R)
        rsqe_bc = sb("rsqe_bc", [128, N])
        wt = [sb(f"wt{b}", [128, KD, F], BF16) for b in range(NWT)]
        wsq = [sb(f"wsq{b}", [128, KD, F], F32R) for b in range(3)]
        lnw_tmp = sb("lnw_tmp", [1, F])
        rsqw_row = [sb(f"rsqw_row{b}", [1, F], F32R) for b in range(2)]
        rsqw_bc = [sb(f"rsqw_bc{b}", [128, F]) for b in range(2)]
        out_sb = [sb(f"out_sb{b}", [128, F]) for b in range(NOUT)]
        ones128 = sb("ones128", [128, 1], F32R)
        ones1 = sb("ones1", [1, 128], F32R)
        ident_sb = sb("ident_sb", [128, 128])
        diag_tmp = sb("diag_tmp", [128, 128])
        vdiag = sb("vdiag", [128, 1])
        sqv = sb("sqv", [128, 1])
        lnu = sb("lnu", [128, 1])
        s3v = sb("s3v", [128, 1])
        t1v = sb("t1v", [128, 1])
        fixp = sb("fixp", [128, 1])
        deltap = sb("deltap", [128, 1])
        lnS_b = sb("lnS_b", [1, 1])
        s2_b = sb("s2_b", [128, 1])

        # --- PSUM: 6 main banks + ssq row bank + broadcast bank = 8 ---
        ps_main = [ps(f"ps_main{b}") for b in range(NPS)]
        ps_ssq = ps("ps_ssq")
        ps_bc = ps("ps_bc")

        # --- semaphores ---
        s_const = sem("s_const")
        s_wt = [sem(f"s_wt{b}") for b in range(NWT)]
        s_dmaout = sem("s_dmaout")
        s_outdone = sem("s_outdone")
        s_sq = sem("s_sq")
        s_ssqmm = sem("s_ssqmm")
        s_row = sem("s_row")
        s_bcmm = sem("s_bcmm")
        s_bcev = sem("s_bcev")
        s_mmtile = sem("s_mmtile")
        s_evtile = sem("s_evtile")
        s_vg = sem("s_vg")
        s_sfix = sem("s_sfix")
        s_vfix = sem("s_vfix")
        s_esq = sem("s_esq")
        s_essqmm = sem("s_essqmm")
        s_erow = sem("s_erow")
        s_ebcmm = sem("s_ebcmm")
        s_ebcp = sem("s_ebcp")
        s_etn = sem("s_etn")

        with nc.Block() as block:

            @block.gpsimd
            def _(g):
                g.memset(lnS_b[:], float(np.log(S)))
                g.memset(s2_b[:], float(S * S))
                def wt_dma(c):
                    g.dma_start(
                        out=wt[c % NWT][:],
                        in_=wT_ext[:, c * F:(c + 1) * F].rearrange("(ko p) f -> p ko f", p=128),
                    ).then_inc(s_wt[c % NWT], 16)

                wt_dma(0)  # first: unblocks squares(0) -> ssq(0) quickly
                g.dma_start(out=ones128[:], in_=ones_ext[:].rearrange("(p o) -> p o", o=1)).then_inc(s_const, 16)
                g.dma_start(out=ones1[:], in_=ones_ext[:].rearrange("(o p) -> o p", o=1)).then_inc(s_const, 16)
                g.dma_start(out=ident_sb[:], in_=ident_ext[:]).then_inc(s_const, 16)
                g.dma_start(out=eT_sb[:], in_=eT_ext[:].rearrange("(ko p) n -> p ko n", p=128)).then_inc(s_const, 16)
                wt_dma(1)
                for c in range(2, min(NWT, NCHUNK)):
                    wt_dma(c)
                for c in range(NCHUNK):
                    cp = c + NWT
                    if cp < NCHUNK:
                        # buffer cp%NWT freed once main(cp-NWT) is done
                        g.wait_ge(s_mmtile, 8 * (cp - NWT + 1))
                        g.dma_start(
                            out=wt[cp % NWT][:],
                            in_=wT_ext[:, cp * F:(cp + 1) * F].rearrange("(ko p) f -> p ko f", p=128),
                        ).then_inc(s_wt[cp % NWT], 16)
                    if c == 0:
                        g.wait_ge(s_vfix, 1)
                    for nb in range(NB):
                        t = c * NB + nb
                        g.wait_ge(s_evtile, t + 1)
                        g.dma_start(
                            out=out_ext[nb * 128:(nb + 1) * 128, c * F:(c + 1) * F],
                            in_=out_sb[t % NOUT][:],
                        ).then_inc(s_dmaout, 16)
                # make sure all output DMAs have landed before the graph ends
                g.wait_ge(s_dmaout, 16 * 8 * NCHUNK)

            @block.scalar
            def _(s):
                def do_squares(x):
                    s.wait_ge(s_wt[x % NWT], 16 * (x // NWT + 1))
                    if x >= 3:
                        s.wait_ge(s_ssqmm, x - 2)  # wsq[x%3] freed by ssq(x-3)
                    for k in range(KD):
                        ins = s.activation(wsq[x % 3][:, k, :], wt[x % NWT][:, k, :], AF.Square)
                    ins.then_inc(s_sq, 1)

                # wT squares for chunks 0/1 as early as possible
                do_squares(0)
                do_squares(1)
                # e-prep
                s.wait_ge(s_const, 64)
                for k in range(KD):
                    ins = s.activation(esq[:, k, :], eT_sb[:, k, :], AF.Square)
                ins.then_inc(s_esq, 1)
                s.wait_ge(s_essqmm, 1)
                for h in range(2):
                    src = ps_main[h][0:1, :]
                    s.activation(lnw_tmp[:], src, AF.Ln)
                    ins = s.activation(rsqe_row[0:1, h * F:(h + 1) * F], lnw_tmp[:], AF.Exp, scale=-0.5, bias=lnS_b[:])
                ins.then_inc(s_erow, 1)
                s.wait_ge(s_ebcmm, 1)
                s.activation(rsqe_bc[:, 0:F], ps_main[2][:], AF.Copy)
                s.activation(rsqe_bc[:, F:N], ps_main[3][:], AF.Copy).then_inc(s_ebcp, 1)
                # chunk loop: bcast-evict first (unblocks VectorE), then
                # squares two chunks ahead, then the rsqw row chain
                for c in range(NCHUNK):
                    if c >= 1:
                        s.wait_ge(s_bcmm, c)
                        s.activation(rsqw_bc[(c - 1) % 2][:], ps_bc[:], AF.Copy).then_inc(s_bcev, 1)
                    if c + 2 <= NCHUNK - 1:
                        do_squares(c + 2)
                    s.wait_ge(s_ssqmm, c + 1)
                    s.activation(lnw_tmp[:], ps_ssq[0:1, :], AF.Ln)
                    s.activation(rsqw_row[c % 2][:], lnw_tmp[:], AF.Exp, scale=-0.5).then_inc(s_row, 1)
                    if c == 1:
                        # margin fix math on the gathered diagonal [128,1]
                        s.wait_ge(s_vg, 1)
                        s.activation(sqv[:], vdiag[:], AF.Square)
                        s.drain()
                        s.activation(lnu[:], sqv[:], AF.Ln, scale=-1.0, bias=s2_b[:])
                        s.drain()
                        s.activation(s3v[:], lnu[:], AF.Exp, scale=0.5)
                        s.activation(t1v[:], vdiag[:], AF.Copy, scale=COSM)
                        s.drain().then_inc(s_sfix, 1)
                # tail: evict last broadcast
                s.wait_ge(s_bcmm, NCHUNK)
                s.activation(rsqw_bc[(NCHUNK - 1) % 2][:], ps_bc[:], AF.Copy).then_inc(s_bcev, 1)

            @block.tensor
            def _(t):
                # chunk-0 ssq as soon as its squares land (keeps PE warm early)
                t.wait_ge(s_sq, 1)
                for k in range(KD):
                    ins = t.matmul(ps_ssq[0:1, :], lhsT=ones128[:], rhs=wsq[0][:, k, :],
                             start=(k == 0), stop=(k == KD - 1))
                ins.then_inc(s_ssqmm, 1)
                # e-prep: ssq_e rows then rsqe broadcast
                t.wait_ge(s_esq, 1)
                for h in range(2):
                    for k in range(KD):
                        ins = t.matmul(
                            ps_main[h][0:1, :], lhsT=ones128[:],
                            rhs=esq[:, k, h * F:(h + 1) * F],
                            start=(k == 0), stop=(k == KD - 1),
                        )
                ins.then_inc(s_essqmm, 1)
                t.wait_ge(s_erow, 1)
                for h in range(2):
                    ins = t.matmul(ps_main[2 + h][:], lhsT=ones1[:],
                             rhs=rsqe_row[0:1, h * F:(h + 1) * F], start=True, stop=True)
                ins.then_inc(s_ebcmm, 1)
                # wait for eTn + eprep psum drains before main work
                t.wait_ge(s_etn, 1)
                t.wait_ge(s_ebcp, 1)
                for B in range(1, NCHUNK + 1):
                    if B <= NCHUNK - 1:
                        # ssq for chunk B
                        t.wait_ge(s_sq, B + 1)
                        t.wait_ge(s_row, B)  # ps_ssq freed by row-chain(B-1)
                        for k in range(KD):
                            ins = t.matmul(ps_ssq[0:1, :], lhsT=ones128[:], rhs=wsq[B % 3][:, k, :],
                                     start=(k == 0), stop=(k == KD - 1))
                        ins.then_inc(s_ssqmm, 1)
                    if B >= 1:
                        A = B - 1
                        # broadcast rsqw(A) to all partitions
                        t.wait_ge(s_row, A + 1)
                        if A >= 1:
                            t.wait_ge(s_bcev, A)  # ps_bc freed by bcevict(A-1)
                        t.matmul(ps_bc[:], lhsT=ones1[:], rhs=rsqw_row[A % 2][:], start=True, stop=True).then_inc(s_bcmm, 1)
                        # main matmuls for chunk A
                        for nb in range(NB):
                            tt = A * NB + nb
                            if tt >= NPS:
                                t.wait_ge(s_evtile, tt - NPS + 1)
                            for k in range(KD):
                                mm = t.matmul(
                                    ps_main[tt % NPS][:],
                                    lhsT=eTn[:, k, nb * 128:(nb + 1) * 128],
                                    rhs=wt[A % NWT][:, k, :],
                                    start=(k == 0), stop=(k == KD - 1),
                                )
                            mm.then_inc(s_mmtile, 1)

            @block.vector
            def _(v):
                # e-prep: eTn = eT * rsqe_bc  (f32r)
                v.wait_ge(s_const, 64)
                v.wait_ge(s_ebcp, 1)
                for k in range(KD):
                    v.tensor_tensor(out=eTn[:, k, :], in0=eT_sb[:, k, :], in1=rsqe_bc[:], op=OP.mult)
                v.engine_nop().then_inc(s_etn, 1)
                for c in range(NCHUNK):
                    for nb in range(NB):
                        tt = c * NB + nb
                        v.wait_ge(s_mmtile, tt + 1)
                        if nb == 0:
                            v.wait_ge(s_bcev, c + 1)
                        if tt >= NOUT:
                            # buffer reused from chunk c-4; DMA completions are
                            # near-FIFO (every DMA sprays all 16 queues)
                            v.wait_ge(s_dmaout, 16 * (tt - NOUT + 8))
                        v.tensor_tensor(out=out_sb[tt % NOUT][:], in0=ps_main[tt % NPS][:],
                                        in1=rsqw_bc[c % 2][:], op=OP.mult)
                        v.engine_nop().then_inc(s_evtile, 1)
                        if c == 0 and nb == 0:
                            # extract the label diagonal of the first tile
                            v.tensor_tensor(out=diag_tmp[:], in0=out_sb[0][:, 0:128],
                                            in1=ident_sb[:], op=OP.mult)
                            v.drain()
                            v.tensor_reduce(vdiag[:], diag_tmp[:],
                                            mybir.AxisListType.X, OP.add)
                            v.drain().then_inc(s_vg, 1)
                    if c == 1:
                        # apply margin on the diagonal of tile (0,0)
                        v.wait_ge(s_sfix, 1)
                        v.scalar_tensor_tensor(fixp[:], s3v[:], -SINM, t1v[:], OP.mult, OP.add)
                        v.drain()
                        v.tensor_tensor(out=deltap[:], in0=fixp[:], in1=vdiag[:], op=OP.subtract)
                        v.drain()
                        v.scalar_tensor_tensor(out_sb[0][:, 0:128], ident_sb[:], deltap[:],
                                               out_sb[0][:, 0:128], OP.mult, OP.add)
                        v.drain().then_inc(s_vfix, 1)

    return nc


_GRAPH = None


def _get_graph():
    global _GRAPH
    if _GRAPH is None:
        _GRAPH = build_graph()
    return _GRAPH


def _host_prepare(embeddings, weight, labels):
    """Row/class permutations putting each core's labels on the (0,0) diagonal."""
    labels = np.asarray(labels).astype(np.int64)
    e = np.asarray(embeddings, dtype=np.float32)
    w = np.asarray(weight, dtype=np.float32)

    # fix instance i (row i, class labels[i]) goes to core i//128, column i%128
    first_seen = {}
    extras = []  # (core, col, row, cls) for duplicate label classes
    primary_col = {}  # cls -> (core, col)
    for i in range(N):
        l = int(labels[i])
        m, p = i // 128, i % 128
        if l not in first_seen:
            first_seen[l] = (m, p)
            primary_col[l] = (m, p)
        else:
            extras.append((m, p, i, l))

    labeled = np.zeros(C, dtype=bool)
    labeled[labels] = True
    unlab = np.nonzero(~labeled)[0]

    # column map per core: -1 = padding column
    colmaps = np.full((NCORES, CS), -1, dtype=np.int64)
    for i in range(N):
        colmaps[i // 128, i % 128] = labels[i]
    fill_slots = NCORES * (CS - 128)
    fill = np.full(fill_slots, -1, dtype=np.int64)
    fill[: unlab.size] = unlab
    fill = fill.reshape(NCORES, CS - 128)
    colmaps[:, 128:] = fill

    # bulk-assign validity: skip pad and non-primary duplicate columns
    valid_bulk = colmaps >= 0
    for (m, p, i, l) in extras:
        valid_bulk[m, p] = False

    wTfull = w.T  # [512, 100000] view
    in_maps = []
    row_perms = []
    ones = np.ones(128, dtype=np.float32)
    ident = np.eye(128, dtype=np.float32)
    for m in range(NCORES):
        cm = colmaps[m]
        wt = np.zeros((D, CS), dtype=np.float32)
        vmask = cm >= 0
        wt[:, vmask] = wTfull[:, cm[vmask]]
        wt[0, ~vmask] = 1.0
        rows = np.concatenate([
            np.arange(m * 128, (m + 1) * 128),
            np.delete(np.arange(N), np.s_[m * 128:(m + 1) * 128]),
        ])
        row_perms.append(rows)
        eT = np.ascontiguousarray(e[rows].T)
        in_maps.append({
            "eT": eT,
            "wT": np.ascontiguousarray(wt),
            "ones": ones,
            "ident": ident,
        })
    return in_maps, row_perms, colmaps, valid_bulk, extras


def _assemble(results, row_perms, colmaps, valid_bulk, extras):
    out = np.empty((N, C), dtype=np.float32)
    slabs = []
    for m in range(NCORES):
        slab = results[m]["out"]
        unperm = np.empty_like(slab)
        unperm[row_perms[m]] = slab
        slabs.append(unperm)
        vb = valid_bulk[m]
        out[:, colmaps[m][vb]] = unperm[:, vb]
    for (m, p, i, l) in extras:
        out[i, l] = slabs[m][i, p]
    return out


def kernel(embeddings, weight, labels, _trace=False):
    nc = _get_graph()
    in_maps, row_perms, colmaps, valid_bulk, extras = _host_prepare(
        embeddings, weight, labels
    )
    res = run_bass_kernel_spmd(nc, in_maps, core_ids=list(range(NCORES)), trace=_trace)
    out = _assemble(res.results, row_perms, colmaps, valid_bulk, extras)
    if _trace:
        return out, res
    return out
